# revision 2
# baseline (speedup 1.0000x reference)
"""Trainium2 Bass kernel for nn_DSC_86071144612259.

The reference network collapses to a single linear contraction

    u[b, c] = sum_{d<128} sum_{p} W[d, p, c] * y_rev[b, d, p]

where W [128, P, MC] is assembled exactly (float64, on host) from the
small parameter tensors.  The 270 MB y_rev stream is the real work and
is purely DMA bound, so the kernel moves y as *int8* (per-batch-row
scale, absmax/127) -- half the HBM traffic of the fp16 baseline -- and
upconverts to fp16 on-chip, spread across the three free compute
engines (DVE / Activation / GpSimd), overlapping the DMA stream.  The
tensor engine chases with fp16 matmuls accumulating in fp32 PSUM; the
per-row dequant scale is applied by the final PSUM->SBUF tensor_mul
(fused with the output copy), so the only numeric loss is the int8
rounding of y (measured absmax-rel ~9.2e-3 < the 2e-2 gate).

Sharding: pure data parallel over the batch axis across 8 cores (2048
rows each); W and the scale tile are replicated per-core inputs.

DMA: y8 is laid out partition-major in DRAM ([128, 65536] int8, each
partition's 64 KB contiguous), so one dma_start covers a multi-chunk
group with long descriptor rows; all y DMAs ride a single ring (sync
engine, ~0.65us issue each), leaving Activation free to convert.
Group sizes ramp up [1,2,3,4,...] so conversion starts on chunk 0
almost immediately.  The int8->fp16 chunk conversions are assigned to
the three engines by a deterministic greedy balance of measured engine
rates against estimated DMA land times.
"""

import numpy as np

B = 16384      # batch
L = 129        # history length of y_rev
P = 32         # observation dim
MC = 16        # control dim (output)
H = 24         # spectral dim
M = 64         # filter length
NCORES = 8
BS = B // NCORES           # 2048 batch rows per core
KD = 128                   # delays with nonzero weight
K = KD * P                 # 4096 contraction length
NKC = K // 128             # 32 k-chunks of 128 partitions
CW = BS                    # SBUF columns per chunk (2048)
NFREE = 512                # matmul moving free dim (one fp32 PSUM bank)
NB = BS // NFREE           # 4 batch chunks per core

# DMA groups over the 32 k-chunks: small first so conversion can start
# immediately, then steady 4-chunk groups (one dma_start each).
GS = [1, 2, 3, 4, 4, 4, 4, 4, 4, 2]
assert sum(GS) == NKC

# ns per [128, 2048] int8->fp16 chunk conversion per engine:
# DVE 0.96 GHz, ACT 1.2 GHz, Pool 1.2 GHz * 0.6 sw efficiency.
CONV_NS = {"vector": 2134, "scalar": 1707, "gpsimd": 2845}
DMA_NS_PER_CHUNK = 794     # 128*2048 B at ~330 GB/s

_CACHE = {}


def _conv_assignment():
    """Greedy assignment of the 32 chunk conversions to the 3 engines,
    balancing engine rates against estimated chunk-arrival times."""
    grp = []
    for g, sz in enumerate(GS):
        grp += [g] * sz
    land = {}
    t = 700.0                      # first dma_start issue latency
    ci = 0
    for sz in GS:
        t += sz * DMA_NS_PER_CHUNK
        for _ in range(sz):
            land[ci] = t
            ci += 1
    free = {e: 0.0 for e in CONV_NS}
    assign = []
    for ci in range(NKC):
        e = min(CONV_NS, key=lambda e: max(free[e], land[ci]) + CONV_NS[e])
        assign.append(e)
        free[e] = max(free[e], land[ci]) + CONV_NS[e]
    return assign, grp


def _build_w(M0, M_tilde, M_0l, M_big, sigma, lambda_e, phi, phi_tilde):
    """Collapse the parameter tensors into W [KD, MC, P] (float64).

    Mirrors reference.py exactly:
      term1: delay 0,      M0
      term2: delays 1..64, sum_i lambda_i^0.25 phi_tilde[j-1,i] M_tilde[i]
      term3: delays 0..63, sum_l sigma_l^0.25  phi[k,l]         M_0l[l]
      term4: delays 1..127 via conv(phi_tilde[:,i], phi[:,l]) and M_big
    """
    f8 = np.float64
    M0 = M0.astype(f8)
    M_tilde = M_tilde.astype(f8)
    M_0l = M_0l.astype(f8)
    M_big = M_big.astype(f8)
    sigma = sigma.astype(f8)
    lambda_e = lambda_e.astype(f8)
    phi = phi.astype(f8)
    phi_tilde = phi_tilde.astype(f8)

    W = np.zeros((KD, MC, P), dtype=f8)
    W[0] += M0
    pt = phi_tilde * (lambda_e ** 0.25)[None, :]
    W[1:M + 1] += np.einsum("ji,icp->jcp", pt, M_tilde)
    ps = phi * (sigma ** 0.25)[None, :]
    W[0:M] += np.einsum("kl,lcp->kcp", ps, M_0l)
    W4 = np.empty((H, H, 2 * M - 1), dtype=f8)
    for i in range(H):
        for l in range(H):
            W4[i, l] = np.convolve(phi_tilde[:, i], phi[:, l])
    scale = (lambda_e[:, None] * sigma[None, :]) ** 0.25
    W[1:2 * M] += np.einsum("ild,ilcp->dcp", W4 * scale[:, :, None], M_big)
    return W


def _get_nc():
    """Build the per-core Bass program (cached)."""
    if "nc" in _CACHE:
        return _CACHE["nc"]
    import concourse.bass as bass
    import concourse.mybir as mybir

    assign, grp = _conv_assignment()
    chunks_of = {e: [ci for ci in range(NKC) if assign[ci] == e]
                 for e in CONV_NS}

    nc = bass.Bass("TRN2", target_bir_lowering=False, enable_partition_id=False)
    y8 = nc.dram_tensor("y8", [128, NKC * CW], mybir.dt.int8, kind="ExternalInput")
    w = nc.dram_tensor("w", [128, NKC * MC], mybir.dt.float16, kind="ExternalInput")
    s = nc.dram_tensor("s", [128, NFREE], mybir.dt.float32, kind="ExternalInput")
    ut = nc.dram_tensor("ut", [128, NFREE], mybir.dt.float16, kind="ExternalOutput")

    y8_sb = nc.alloc_sbuf_tensor("y8_sb", [128, NKC * CW], mybir.dt.int8)
    y_sb = nc.alloc_sbuf_tensor("y_sb", [128, NKC * CW], mybir.dt.float16)
    # W pre-swizzled on host: w_sb[p, ki*MC + c] = W_flat[ki*128 + p, c]
    w_sb = nc.alloc_sbuf_tensor("w_sb", [128, NKC * MC], mybir.dt.float16)
    # Dequant tile: s_sb[32*bc + c, j] = s_row[bc*512 + j]
    s_sb = nc.alloc_sbuf_tensor("s_sb", [128, NFREE], mybir.dt.float32)
    # Output striped across partitions: row 32*bc + c holds u^T[c, bc*512+j]
    u_sb = nc.alloc_sbuf_tensor("u_sb", [128, NFREE], mybir.dt.float16)
    ps = nc.alloc_psum_tensor("ps", [128, NFREE], mybir.dt.float32)

    sem_yg = [nc.alloc_semaphore(f"sem_yg{g}") for g in range(len(GS))]
    sem_w = nc.alloc_semaphore("sem_w")
    sem_s = nc.alloc_semaphore("sem_s")
    sem_cv = {e: nc.alloc_semaphore(f"sem_cv_{e}") for e in CONV_NS}
    pe_done = nc.alloc_semaphore("pe_done")
    ve_done = nc.alloc_semaphore("ve_done")
    odma = nc.alloc_semaphore("odma")

    goff = [0]
    for sz in GS:
        goff.append(goff[-1] + sz)

    def conv_ops(eng, ename):
        lastg = -1
        for ci in chunks_of[ename]:
            if grp[ci] != lastg:
                eng.wait_ge(sem_yg[grp[ci]], 16)
                lastg = grp[ci]
            lo, hi = ci * CW, (ci + 1) * CW
            if ename == "scalar":
                op = eng.copy(out=y_sb[:, lo:hi], in_=y8_sb[:, lo:hi])
            else:
                op = eng.tensor_copy(out=y_sb[:, lo:hi], in_=y8_sb[:, lo:hi])
            op.then_inc(sem_cv[ename], 1)

    with nc.Block() as block:

        @block.sync
        def _(sync):
            # Single ring: y group 0 first (start streaming ASAP), then
            # the tiny W / scale tiles, then the remaining groups.
            sync.dma_start(
                out=y8_sb[:, goff[0] * CW:goff[1] * CW],
                in_=y8[:, goff[0] * CW:goff[1] * CW],
            ).then_inc(sem_yg[0], 16)
            sync.dma_start(out=w_sb[:, :], in_=w[:, :]).then_inc(sem_w, 16)
            sync.dma_start(out=s_sb[:, :], in_=s[:, :]).then_inc(sem_s, 16)
            for g in range(1, len(GS)):
                sync.dma_start(
                    out=y8_sb[:, goff[g] * CW:goff[g + 1] * CW],
                    in_=y8[:, goff[g] * CW:goff[g + 1] * CW],
                ).then_inc(sem_yg[g], 16)
            sync.wait_ge(ve_done, 1)
            sync.dma_start(
                out=ut[:, :NFREE // 2], in_=u_sb[:, :NFREE // 2]
            ).then_inc(odma, 16)
            sync.wait_ge(odma, 32)

        @block.scalar
        def _(scalar):
            conv_ops(scalar, "scalar")
            scalar.wait_ge(ve_done, 2)
            scalar.dma_start(
                out=ut[:, NFREE // 2:], in_=u_sb[:, NFREE // 2:]
            ).then_inc(odma, 16)
            scalar.wait_ge(odma, 32)

        @block.gpsimd
        def _(gpsimd):
            conv_ops(gpsimd, "gpsimd")

        @block.tensor
        def _(tensor):
            tensor.wait_ge(sem_w, 16)
            cnt = {e: 0 for e in CONV_NS}
            for ci in range(NKC - 1):
                e = assign[ci]
                cnt[e] += 1
                tensor.wait_ge(sem_cv[e], cnt[e])
                for bc in range(NB):
                    tensor.matmul(
                        ps[32 * bc:32 * bc + MC, :],
                        w_sb[:, ci * MC:(ci + 1) * MC],
                        y_sb[:, ci * CW + bc * NFREE:ci * CW + (bc + 1) * NFREE],
                        start=(ci == 0),
                        stop=False,
                        tile_position=(0, 32 * bc),
                    )
            # Last chunk in two N=256 halves so the dequant+store of the
            # first half overlaps the second half's matmuls.
            ci = NKC - 1
            e = assign[ci]
            cnt[e] += 1
            tensor.wait_ge(sem_cv[e], cnt[e])
            for half in range(2):
                lo, hi = half * NFREE // 2, (half + 1) * NFREE // 2
                for bc in range(NB):
                    mm = tensor.matmul(
                        ps[32 * bc:32 * bc + MC, lo:hi],
                        w_sb[:, ci * MC:(ci + 1) * MC],
                        y_sb[:, ci * CW + bc * NFREE + lo:ci * CW + bc * NFREE + hi],
                        start=False,
                        stop=True,
                        tile_position=(0, 32 * bc),
                    )
                    mm.then_inc(pe_done, 1)

        @block.vector
        def _(vector):
            conv_ops(vector, "vector")
            vector.wait_ge(sem_s, 16)
            for half in range(2):
                lo, hi = half * NFREE // 2, (half + 1) * NFREE // 2
                vector.wait_ge(pe_done, NB * (half + 1))
                vector.tensor_mul(
                    out=u_sb[:, lo:hi], in0=ps[:, lo:hi], in1=s_sb[:, lo:hi]
                ).then_inc(ve_done, 1)

    _CACHE["nc"] = nc
    return nc


def _ensure_ntff_hook():
    """bass_utils hard-imports antenv.axon_hooks when BASS_TRACE is set;
    this container's trimmed antenv lacks it.  Register a working stub
    built from trn_agent_boot's ctypes NTFF driver (or a None hook,
    which bass_utils degrades gracefully on)."""
    import importlib.util
    import sys
    import types

    if "antenv.axon_hooks" in sys.modules:
        return
    try:
        if importlib.util.find_spec("antenv.axon_hooks") is not None:
            return
    except (ImportError, ValueError):
        pass
    try:
        from trn_agent_boot.trn_boot import _ntff_profile_via_ctypes

        hook = _ntff_profile_via_ctypes("/opt/axon/libaxon_pjrt.so")
    except Exception:
        hook = None
    mod = types.ModuleType("antenv.axon_hooks")
    mod.get_axon_ntff_profile_hook = lambda: hook
    sys.modules["antenv.axon_hooks"] = mod


def kernel(y_rev, M0, M_tilde, M_0l, M_big, sigma, lambda_e, phi, phi_tilde):
    _ensure_ntff_hook()
    from concourse.bass_utils import run_bass_kernel_spmd

    W = _build_w(M0, M_tilde, M_0l, M_big, sigma, lambda_e, phi, phi_tilde)
    # W_flat[k, c] with k = d*P + p, then swizzled so chunk ki sits at
    # columns [ki*MC, (ki+1)*MC) of a [128, NKC*MC] tile.
    Wf = W.transpose(0, 2, 1).reshape(K, MC)
    Wd = np.ascontiguousarray(
        Wf.reshape(NKC, 128, MC).transpose(1, 0, 2).reshape(128, NKC * MC)
    ).astype(np.float16)

    in_maps = []
    for sh in range(NCORES):
        blk = y_rev[sh * BS:(sh + 1) * BS, :KD, :].reshape(BS, K)  # [b, k]
        srow = (np.abs(blk).max(axis=1) / 127.0).astype(np.float32)  # [BS]
        np.maximum(srow, 1e-30, out=srow)
        q = np.rint(blk / srow[:, None])
        np.clip(q, -127, 127, out=q)
        q = q.astype(np.int8)
        # partition-major DRAM layout: y8[p, ki*CW + j] = q[j, ki*128 + p]
        ytp = np.ascontiguousarray(
            q.T.reshape(NKC, 128, CW).transpose(1, 0, 2).reshape(128, NKC * CW)
        )
        stile = np.empty((128, NFREE), dtype=np.float32)
        for bc in range(NB):
            stile[32 * bc:32 * (bc + 1), :] = srow[None, bc * NFREE:(bc + 1) * NFREE]
        in_maps.append({"y8": ytp, "w": Wd, "s": stile})

    res = run_bass_kernel_spmd(_get_nc(), in_maps, list(range(NCORES)))
    _CACHE["last_result"] = res

    out = np.empty((B, MC), dtype=np.float32)
    for sh in range(NCORES):
        # ut[32*bc + c, j] = u^T[c, bc*512 + j]
        stripes = res.results[sh]["ut"].reshape(NB, 32, NFREE)[:, :MC, :]
        out[sh * BS:(sh + 1) * BS, :] = (
            stripes.transpose(0, 2, 1).reshape(BS, MC).astype(np.float32)
        )
    return out


# revision 5
# speedup vs baseline: 1.5134x; 1.5134x over previous
"""Trainium2 Bass kernel for nn_DSC_86071144612259.

The reference network collapses to a single linear contraction

    u[b, c] = sum_{d<128} sum_{p} W[d, p, c] * y_rev[b, d, p]

where W [128, P, MC] is assembled exactly (float64, on host) from the
small parameter tensors.  The 270 MB y_rev stream is the real work and
is purely DMA bound, so the kernel moves y as *int8* (per-batch-row
scale, absmax/127) -- half the HBM traffic of the fp16 baseline.  The
PE only eats float dtypes (the BIR verifier rejects integer matmuls),
so int8 y is upconverted to fp16 on-chip via two paths that overlap
the stream:

  * chunks 0..11: gpsimd software-DGE *casting DMAs* -- the DMA engines
    convert int8->fp16 in flight (measured ~423 GB/s SBUF-side, i.e.
    ~212 GB/s HBM-side),
  * chunks 12..31: plain int8 DMAs on the sync HWDGE ring (measured
    ~375 GB/s with 1 MB descriptors), then tensor_copy / activation-
    Copy casts split between DVE (~1.22 us/chunk) and ACT (~2.0
    us/chunk).  GpSimd tensor_copy casts are NOT used: they run ~8 us
    and drag concurrent DVE casts down to the same pace (measured).

The tensor engine chases per chunk with fp16 matmuls accumulating in
fp32 PSUM (4 batch blocks concurrently in disjoint 32-column PE
groups); the per-row dequant scale is applied by the final PSUM->SBUF
tensor_mul, fused with the output copy.  The only numeric loss is the
int8 rounding of y (measured absmax-rel ~9.2e-3 < the 2e-2 gate).

Sharding: pure data parallel over the batch axis across 8 cores (2048
rows each); W and the scale tile are replicated per-core inputs.
"""

import numpy as np

B = 16384      # batch
L = 129        # history length of y_rev
P = 32         # observation dim
MC = 16        # control dim (output)
H = 24         # spectral dim
M = 64         # filter length
NCORES = 8
BS = B // NCORES           # 2048 batch rows per core
KD = 128                   # delays with nonzero weight
K = KD * P                 # 4096 contraction length
NKC = K // 128             # 32 k-chunks of 128 partitions
CW = BS                    # SBUF columns per chunk (2048)
NFREE = 512                # matmul moving free dim (one fp32 PSUM bank)
NB = BS // NFREE           # 4 batch chunks per core

# Chunks 0..NCAST-1 arrive via gpsimd casting DMAs (groups of 3);
# chunks NCAST..31 arrive int8 on the sync ring (groups of 4) and are
# cast by DVE/ACT.
NCAST = 12
CAST_GROUPS = [[0, 1, 2], [3, 4, 5], [6, 7, 8], [9, 10, 11]]
SYNC_GROUPS = [[12, 13, 14, 15], [16, 17, 18, 19], [20, 21, 22, 23],
               [24, 25, 26, 27], [28, 29, 30, 31]]

# Measured (HW trace) ns per [128, 2048] int8->fp16 chunk cast.
CONV_NS = {"vector": 1225, "scalar": 1990}

_CACHE = {}


def _conv_assignment():
    """Greedy DVE/ACT split of chunks NCAST..31, balancing measured
    rates against estimated sync-ring arrival times.  Chunk 31 is
    forced onto DVE (faster) to shorten the tail."""
    grp = {}
    for gi, chunks in enumerate(CAST_GROUPS):
        for ci in chunks:
            grp[ci] = ("cast", gi)
    land = {}
    t = 1400.0
    for gi, chunks in enumerate(SYNC_GROUPS):
        # sync ring ~150-375 GB/s while sharing HBM with the cast queue
        t += len(chunks) * 1100.0
        for ci in chunks:
            grp[ci] = ("sync", gi)
            land[ci] = t
    free = {"vector": 0.0, "scalar": 1300.0}   # ACT pays a one-time table load
    assign = {}
    for ci in range(NCAST, NKC):
        if ci == NKC - 1:
            e = "vector"
        else:
            e = min(CONV_NS, key=lambda e: max(free[e], land[ci]) + CONV_NS[e])
        assign[ci] = e
        free[e] = max(free[e], land[ci]) + CONV_NS[e]
    return assign, grp


def _build_w(M0, M_tilde, M_0l, M_big, sigma, lambda_e, phi, phi_tilde):
    """Collapse the parameter tensors into W [KD, MC, P] (float64).

    Mirrors reference.py exactly:
      term1: delay 0,      M0
      term2: delays 1..64, sum_i lambda_i^0.25 phi_tilde[j-1,i] M_tilde[i]
      term3: delays 0..63, sum_l sigma_l^0.25  phi[k,l]         M_0l[l]
      term4: delays 1..127 via conv(phi_tilde[:,i], phi[:,l]) and M_big
    """
    f8 = np.float64
    M0 = M0.astype(f8)
    M_tilde = M_tilde.astype(f8)
    M_0l = M_0l.astype(f8)
    M_big = M_big.astype(f8)
    sigma = sigma.astype(f8)
    lambda_e = lambda_e.astype(f8)
    phi = phi.astype(f8)
    phi_tilde = phi_tilde.astype(f8)

    W = np.zeros((KD, MC, P), dtype=f8)
    W[0] += M0
    pt = phi_tilde * (lambda_e ** 0.25)[None, :]
    W[1:M + 1] += np.einsum("ji,icp->jcp", pt, M_tilde)
    ps = phi * (sigma ** 0.25)[None, :]
    W[0:M] += np.einsum("kl,lcp->kcp", ps, M_0l)
    W4 = np.empty((H, H, 2 * M - 1), dtype=f8)
    for i in range(H):
        for l in range(H):
            W4[i, l] = np.convolve(phi_tilde[:, i], phi[:, l])
    scale = (lambda_e[:, None] * sigma[None, :]) ** 0.25
    W[1:2 * M] += np.einsum("ild,ilcp->dcp", W4 * scale[:, :, None], M_big)
    return W


def _get_nc():
    """Build the per-core Bass program (cached)."""
    if "nc" in _CACHE:
        return _CACHE["nc"]
    import concourse.bass as bass
    import concourse.mybir as mybir

    assign, grp = _conv_assignment()
    chunks_of = {e: [ci for ci in sorted(assign) if assign[ci] == e]
                 for e in CONV_NS}

    nc = bass.Bass("TRN2", target_bir_lowering=False, enable_partition_id=False)
    y8 = nc.dram_tensor("y8", [128, NKC * CW], mybir.dt.int8, kind="ExternalInput")
    w = nc.dram_tensor("w", [128, NKC * MC], mybir.dt.float16, kind="ExternalInput")
    s = nc.dram_tensor("s", [128, NFREE], mybir.dt.float32, kind="ExternalInput")
    ut = nc.dram_tensor("ut", [128, NFREE], mybir.dt.float16, kind="ExternalOutput")

    # int8 staging only for the sync-ring chunks; cast-DMA chunks land
    # directly in y_sb as fp16.
    y8_sb = nc.alloc_sbuf_tensor("y8_sb", [128, (NKC - NCAST) * CW], mybir.dt.int8)
    y_sb = nc.alloc_sbuf_tensor("y_sb", [128, NKC * CW], mybir.dt.float16)
    # W pre-swizzled on host: w_sb[p, ki*MC + c] = W_flat[ki*128 + p, c]
    w_sb = nc.alloc_sbuf_tensor("w_sb", [128, NKC * MC], mybir.dt.float16)
    # Dequant tile: s_sb[32*bc + c, j] = s_row[bc*512 + j]
    s_sb = nc.alloc_sbuf_tensor("s_sb", [128, NFREE], mybir.dt.float32)
    # Output striped across partitions: row 32*bc + c holds u^T[c, bc*512+j]
    u_sb = nc.alloc_sbuf_tensor("u_sb", [128, NFREE], mybir.dt.float16)
    ps = nc.alloc_psum_tensor("ps", [128, NFREE], mybir.dt.float32)

    sem_cg = [nc.alloc_semaphore(f"sem_cg{g}") for g in range(len(CAST_GROUPS))]
    sem_sg = [nc.alloc_semaphore(f"sem_sg{g}") for g in range(len(SYNC_GROUPS))]
    sem_w = nc.alloc_semaphore("sem_w")
    sem_s = nc.alloc_semaphore("sem_s")
    sem_cv = {e: nc.alloc_semaphore(f"sem_cv_{e}") for e in CONV_NS}
    pe_done = nc.alloc_semaphore("pe_done")
    ve_done = nc.alloc_semaphore("ve_done")
    odma = nc.alloc_semaphore("odma")

    def conv_ops(eng, ename):
        lastg = None
        for ci in chunks_of[ename]:
            if grp[ci] != lastg:
                eng.wait_ge(sem_sg[grp[ci][1]], 16)
                lastg = grp[ci]
            src_lo = (ci - NCAST) * CW
            lo = ci * CW
            if ename == "scalar":
                op = eng.copy(out=y_sb[:, lo:lo + CW],
                              in_=y8_sb[:, src_lo:src_lo + CW])
            else:
                op = eng.tensor_copy(out=y_sb[:, lo:lo + CW],
                                     in_=y8_sb[:, src_lo:src_lo + CW])
            op.then_inc(sem_cv[ename], 1)

    with nc.Block() as block:

        @block.sync
        def _(sync):
            for g, chunks in enumerate(SYNC_GROUPS):
                dlo = (chunks[0] - NCAST) * CW
                dhi = (chunks[-1] + 1 - NCAST) * CW
                slo, shi = chunks[0] * CW, (chunks[-1] + 1) * CW
                sync.dma_start(
                    out=y8_sb[:, dlo:dhi], in_=y8[:, slo:shi]
                ).then_inc(sem_sg[g], 16)
            sync.wait_ge(ve_done, 1)
            sync.dma_start(
                out=ut[:, :NFREE // 2], in_=u_sb[:, :NFREE // 2]
            ).then_inc(odma, 16)
            sync.wait_ge(odma, 32)

        @block.gpsimd
        def _(gpsimd):
            # casting DMAs: DRAM int8 -> SBUF fp16, converted in flight
            for g, chunks in enumerate(CAST_GROUPS):
                lo, hi = chunks[0] * CW, (chunks[-1] + 1) * CW
                gpsimd.dma_start(
                    out=y_sb[:, lo:hi], in_=y8[:, lo:hi]
                ).then_inc(sem_cg[g], 16)

        @block.scalar
        def _(scalar):
            # W first (tensor engine blocks on it), then the scale tile.
            scalar.dma_start(out=w_sb[:, :], in_=w[:, :]).then_inc(sem_w, 16)
            scalar.dma_start(out=s_sb[:, :], in_=s[:, :]).then_inc(sem_s, 16)
            conv_ops(scalar, "scalar")
            scalar.wait_ge(ve_done, 2)
            scalar.dma_start(
                out=ut[:, NFREE // 2:], in_=u_sb[:, NFREE // 2:]
            ).then_inc(odma, 16)
            scalar.wait_ge(odma, 32)

        @block.tensor
        def _(tensor):
            tensor.wait_ge(sem_w, 16)
            cnt = {e: 0 for e in CONV_NS}

            def wait_chunk(ci):
                if ci < NCAST:
                    g = grp[ci][1]
                    tensor.wait_ge(sem_cg[g], 16)
                else:
                    e = assign[ci]
                    cnt[e] += 1
                    tensor.wait_ge(sem_cv[e], cnt[e])

            for ci in range(NKC - 1):
                wait_chunk(ci)
                for bc in range(NB):
                    tensor.matmul(
                        ps[32 * bc:32 * bc + MC, :],
                        w_sb[:, ci * MC:(ci + 1) * MC],
                        y_sb[:, ci * CW + bc * NFREE:ci * CW + (bc + 1) * NFREE],
                        start=(ci == 0),
                        stop=False,
                        tile_position=(0, 32 * bc),
                    )
            # Last chunk in two N=256 halves so the dequant+store of the
            # first half overlaps the second half's matmuls.
            ci = NKC - 1
            wait_chunk(ci)
            for half in range(2):
                lo, hi = half * NFREE // 2, (half + 1) * NFREE // 2
                for bc in range(NB):
                    mm = tensor.matmul(
                        ps[32 * bc:32 * bc + MC, lo:hi],
                        w_sb[:, ci * MC:(ci + 1) * MC],
                        y_sb[:, ci * CW + bc * NFREE + lo:ci * CW + bc * NFREE + hi],
                        start=False,
                        stop=True,
                        tile_position=(0, 32 * bc),
                    )
                    mm.then_inc(pe_done, 1)

        @block.vector
        def _(vector):
            conv_ops(vector, "vector")
            vector.wait_ge(sem_s, 16)
            for half in range(2):
                lo, hi = half * NFREE // 2, (half + 1) * NFREE // 2
                vector.wait_ge(pe_done, NB * (half + 1))
                vector.tensor_mul(
                    out=u_sb[:, lo:hi], in0=ps[:, lo:hi], in1=s_sb[:, lo:hi]
                ).then_inc(ve_done, 1)

    _CACHE["nc"] = nc
    return nc


def _ensure_ntff_hook():
    """bass_utils hard-imports antenv.axon_hooks when BASS_TRACE is set;
    this container's trimmed antenv lacks it.  Register a working stub
    built from trn_agent_boot's ctypes NTFF driver (or a None hook,
    which bass_utils degrades gracefully on)."""
    import importlib.util
    import sys
    import types

    if "antenv.axon_hooks" in sys.modules:
        return
    try:
        if importlib.util.find_spec("antenv.axon_hooks") is not None:
            return
    except (ImportError, ValueError):
        pass
    try:
        from trn_agent_boot.trn_boot import _ntff_profile_via_ctypes

        hook = _ntff_profile_via_ctypes("/opt/axon/libaxon_pjrt.so")
    except Exception:
        hook = None
    mod = types.ModuleType("antenv.axon_hooks")
    mod.get_axon_ntff_profile_hook = lambda: hook
    sys.modules["antenv.axon_hooks"] = mod


def kernel(y_rev, M0, M_tilde, M_0l, M_big, sigma, lambda_e, phi, phi_tilde):
    _ensure_ntff_hook()
    from concourse.bass_utils import run_bass_kernel_spmd

    W = _build_w(M0, M_tilde, M_0l, M_big, sigma, lambda_e, phi, phi_tilde)
    # W_flat[k, c] with k = d*P + p, then swizzled so chunk ki sits at
    # columns [ki*MC, (ki+1)*MC) of a [128, NKC*MC] tile.
    Wf = W.transpose(0, 2, 1).reshape(K, MC)
    Wd = np.ascontiguousarray(
        Wf.reshape(NKC, 128, MC).transpose(1, 0, 2).reshape(128, NKC * MC)
    ).astype(np.float16)

    in_maps = []
    for sh in range(NCORES):
        blk = y_rev[sh * BS:(sh + 1) * BS, :KD, :].reshape(BS, K)  # [b, k]
        srow = (np.abs(blk).max(axis=1) / 127.0).astype(np.float32)  # [BS]
        np.maximum(srow, 1e-30, out=srow)
        q = np.rint(blk / srow[:, None])
        np.clip(q, -127, 127, out=q)
        q = q.astype(np.int8)
        # partition-major DRAM layout: y8[p, ki*CW + j] = q[j, ki*128 + p]
        ytp = np.ascontiguousarray(
            q.T.reshape(NKC, 128, CW).transpose(1, 0, 2).reshape(128, NKC * CW)
        )
        stile = np.empty((128, NFREE), dtype=np.float32)
        for bc in range(NB):
            stile[32 * bc:32 * (bc + 1), :] = srow[None, bc * NFREE:(bc + 1) * NFREE]
        in_maps.append({"y8": ytp, "w": Wd, "s": stile})

    res = run_bass_kernel_spmd(_get_nc(), in_maps, list(range(NCORES)))
    _CACHE["last_result"] = res

    out = np.empty((B, MC), dtype=np.float32)
    for sh in range(NCORES):
        # ut[32*bc + c, j] = u^T[c, bc*512 + j]
        stripes = res.results[sh]["ut"].reshape(NB, 32, NFREE)[:, :MC, :]
        out[sh * BS:(sh + 1) * BS, :] = (
            stripes.transpose(0, 2, 1).reshape(BS, MC).astype(np.float32)
        )
    return out


# revision 11
# speedup vs baseline: 1.7740x; 1.1722x over previous
"""Trainium2 Bass kernel for nn_DSC_86071144612259.

The reference network collapses to a single linear contraction

    u[b, c] = sum_{d<128} sum_{p} W[d, p, c] * y_rev[b, d, p]

where W [128, P, MC] is assembled exactly (float64, on host) from the
small parameter tensors.  The 270 MB y_rev stream is the real work and
is purely DMA bound, so the kernel moves y as *int8* (per-batch-row
scale, absmax/127) -- half the HBM traffic of the fp16 baseline.  The
PE only eats float dtypes (the BIR verifier rejects integer matmuls),
so int8 y is upconverted to fp16 on-chip: the sync HWDGE ring streams
int8 (measured ~375 GB/s with 2 MB descriptors, the per-core HBM
share), and the casts are split between DVE tensor_copy (~1.22
us/chunk) and ACT activation-Copy (~2.0 us/chunk), which together
(~1.32 chunks/us) hide under the stream (~0.7 us/chunk).

The tensor engine chases per chunk with fp16 matmuls accumulating in
fp32 PSUM (4 batch blocks concurrently in disjoint 32-column PE
groups); the per-row dequant scale is applied by the final PSUM->SBUF
tensor_mul, fused with the output copy.  The only numeric loss is the
int8 rounding of y (measured absmax-rel ~9.2e-3 < the 2e-2 gate).

Sharding: pure data parallel over the batch axis across 8 cores (2048
rows each); W and the scale tile are replicated per-core inputs.
"""

import numpy as np

B = 16384      # batch
L = 129        # history length of y_rev
P = 32         # observation dim
MC = 16        # control dim (output)
H = 24         # spectral dim
M = 64         # filter length
NCORES = 8
BS = B // NCORES           # 2048 batch rows per core
KD = 128                   # delays with nonzero weight
K = KD * P                 # 4096 contraction length
NKC = K // 128             # 32 k-chunks of 128 partitions
CW = BS                    # SBUF columns per chunk (2048)
NFREE = 512                # matmul moving free dim (one fp32 PSUM bank)
NB = BS // NFREE           # 4 batch chunks per core

# All 32 chunks arrive int8 on the sync HWDGE ring and are cast to
# fp16 by DVE/ACT.  Group sizes ramp 1->8->1: fine granularity at the
# head (casts start on chunk 0 immediately) and tail (short critical
# path after the last byte), 2 MB descriptors in the bulk (the ring
# only reaches ~375 GB/s with large transfers).
SYNC_GROUPS = [[0], [1, 2], [3, 4, 5, 6], [7, 8, 9, 10, 11, 12, 13, 14],
               [15, 16, 17, 18, 19, 20, 21, 22], [23, 24, 25, 26],
               [27, 28], [29, 30], [31]]

# Measured (HW trace) ns per [128, 2048] int8->fp16 chunk cast.
# GpSimd casts are NOT used: ~8 us/chunk AND they drag concurrent DVE
# casts down to the same pace (measured).  GpSimd *casting DMAs* are
# also out: run concurrently with the plain HWDGE stream, both queues
# collapse (113+95 GB/s vs 375/212 solo).
CONV_NS = {"vector": 1225, "scalar": 1990}
_CACHE = {}


def _conv_assignment():
    """Greedy DVE/ACT split of the 32 chunk casts, balancing measured
    rates against estimated sync-ring arrival times.  Chunk 31 is
    forced onto DVE (faster) to shorten the tail."""
    grp = {}
    land = {}
    t = 1400.0
    for gi, chunks in enumerate(SYNC_GROUPS):
        t += len(chunks) * 700.0          # ring ~375 GB/s
        for ci in chunks:
            grp[ci] = gi
            land[ci] = t
    free = {"vector": 0.0, "scalar": 1300.0}   # ACT pays a one-time table load
    assign = {}
    for ci in range(NKC):
        if ci == NKC - 1:
            e = "vector"
        else:
            e = min(CONV_NS, key=lambda e: max(free[e], land[ci]) + CONV_NS[e])
        assign[ci] = e
        free[e] = max(free[e], land[ci]) + CONV_NS[e]
    return assign, grp


def _build_w(M0, M_tilde, M_0l, M_big, sigma, lambda_e, phi, phi_tilde):
    """Collapse the parameter tensors into W [KD, MC, P] (float64).

    Mirrors reference.py exactly:
      term1: delay 0,      M0
      term2: delays 1..64, sum_i lambda_i^0.25 phi_tilde[j-1,i] M_tilde[i]
      term3: delays 0..63, sum_l sigma_l^0.25  phi[k,l]         M_0l[l]
      term4: delays 1..127 via conv(phi_tilde[:,i], phi[:,l]) and M_big
    """
    f8 = np.float64
    M0 = M0.astype(f8)
    M_tilde = M_tilde.astype(f8)
    M_0l = M_0l.astype(f8)
    M_big = M_big.astype(f8)
    sigma = sigma.astype(f8)
    lambda_e = lambda_e.astype(f8)
    phi = phi.astype(f8)
    phi_tilde = phi_tilde.astype(f8)

    W = np.zeros((KD, MC, P), dtype=f8)
    W[0] += M0
    pt = phi_tilde * (lambda_e ** 0.25)[None, :]
    W[1:M + 1] += np.einsum("ji,icp->jcp", pt, M_tilde)
    ps = phi * (sigma ** 0.25)[None, :]
    W[0:M] += np.einsum("kl,lcp->kcp", ps, M_0l)
    W4 = np.empty((H, H, 2 * M - 1), dtype=f8)
    for i in range(H):
        for l in range(H):
            W4[i, l] = np.convolve(phi_tilde[:, i], phi[:, l])
    scale = (lambda_e[:, None] * sigma[None, :]) ** 0.25
    W[1:2 * M] += np.einsum("ild,ilcp->dcp", W4 * scale[:, :, None], M_big)
    return W


def _get_nc():
    """Build the per-core Bass program (cached)."""
    if "nc" in _CACHE:
        return _CACHE["nc"]
    import concourse.bass as bass
    import concourse.mybir as mybir

    assign, grp = _conv_assignment()
    chunks_of = {e: [ci for ci in sorted(assign) if assign[ci] == e]
                 for e in CONV_NS}

    nc = bass.Bass("TRN2", target_bir_lowering=False, enable_partition_id=False)
    y8 = nc.dram_tensor("y8", [128, NKC * CW], mybir.dt.int8, kind="ExternalInput")
    w = nc.dram_tensor("w", [128, NKC * MC], mybir.dt.float16, kind="ExternalInput")
    s = nc.dram_tensor("s", [128, NFREE], mybir.dt.float32, kind="ExternalInput")
    ut = nc.dram_tensor("ut", [128, NFREE], mybir.dt.float16, kind="ExternalOutput")

    y8_sb = nc.alloc_sbuf_tensor("y8_sb", [128, NKC * CW], mybir.dt.int8)
    y_sb = nc.alloc_sbuf_tensor("y_sb", [128, NKC * CW], mybir.dt.float16)
    # W pre-swizzled on host: w_sb[p, ki*MC + c] = W_flat[ki*128 + p, c]
    w_sb = nc.alloc_sbuf_tensor("w_sb", [128, NKC * MC], mybir.dt.float16)
    # Dequant tile: s_sb[32*bc + c, j] = s_row[bc*512 + j]
    s_sb = nc.alloc_sbuf_tensor("s_sb", [128, NFREE], mybir.dt.float32)
    # Output striped across partitions: row 32*bc + c holds u^T[c, bc*512+j]
    u_sb = nc.alloc_sbuf_tensor("u_sb", [128, NFREE], mybir.dt.float16)
    ps = nc.alloc_psum_tensor("ps", [128, NFREE], mybir.dt.float32)

    sem_sg = [nc.alloc_semaphore(f"sem_sg{g}") for g in range(len(SYNC_GROUPS))]
    sem_w = nc.alloc_semaphore("sem_w")
    sem_s = nc.alloc_semaphore("sem_s")
    sem_cv = {e: nc.alloc_semaphore(f"sem_cv_{e}") for e in CONV_NS}
    pe_done = nc.alloc_semaphore("pe_done")
    ve_done = nc.alloc_semaphore("ve_done")
    odma = nc.alloc_semaphore("odma")

    def conv_ops(eng, ename):
        lastg = None
        for ci in chunks_of[ename]:
            if grp[ci] != lastg:
                eng.wait_ge(sem_sg[grp[ci]], 16)
                lastg = grp[ci]
            lo = ci * CW
            if ename == "scalar":
                op = eng.copy(out=y_sb[:, lo:lo + CW],
                              in_=y8_sb[:, lo:lo + CW])
            else:
                op = eng.tensor_copy(out=y_sb[:, lo:lo + CW],
                                     in_=y8_sb[:, lo:lo + CW])
            op.then_inc(sem_cv[ename], 1)

    with nc.Block() as block:

        @block.sync
        def _(sync):
            for g, chunks in enumerate(SYNC_GROUPS):
                lo, hi = chunks[0] * CW, (chunks[-1] + 1) * CW
                sync.dma_start(
                    out=y8_sb[:, lo:hi], in_=y8[:, lo:hi]
                ).then_inc(sem_sg[g], 16)
            sync.wait_ge(ve_done, 1)
            sync.dma_start(
                out=ut[:, :NFREE // 2], in_=u_sb[:, :NFREE // 2]
            ).then_inc(odma, 16)
            sync.wait_ge(odma, 32)

        @block.scalar
        def _(scalar):
            # W first (tensor engine blocks on it), then the scale tile.
            scalar.dma_start(out=w_sb[:, :], in_=w[:, :]).then_inc(sem_w, 16)
            scalar.dma_start(out=s_sb[:, :], in_=s[:, :]).then_inc(sem_s, 16)
            conv_ops(scalar, "scalar")
            scalar.wait_ge(ve_done, 2)
            scalar.dma_start(
                out=ut[:, NFREE // 2:], in_=u_sb[:, NFREE // 2:]
            ).then_inc(odma, 16)
            scalar.wait_ge(odma, 32)

        @block.tensor
        def _(tensor):
            tensor.wait_ge(sem_w, 16)
            cnt = {e: 0 for e in CONV_NS}

            def wait_chunk(ci):
                e = assign[ci]
                cnt[e] += 1
                tensor.wait_ge(sem_cv[e], cnt[e])

            for ci in range(NKC - 1):
                wait_chunk(ci)
                for bc in range(NB):
                    tensor.matmul(
                        ps[32 * bc:32 * bc + MC, :],
                        w_sb[:, ci * MC:(ci + 1) * MC],
                        y_sb[:, ci * CW + bc * NFREE:ci * CW + (bc + 1) * NFREE],
                        start=(ci == 0),
                        stop=False,
                        tile_position=(0, 32 * bc),
                    )
            # Last chunk in two N=256 halves so the dequant+store of the
            # first half overlaps the second half's matmuls.
            ci = NKC - 1
            wait_chunk(ci)
            for half in range(2):
                lo, hi = half * NFREE // 2, (half + 1) * NFREE // 2
                for bc in range(NB):
                    mm = tensor.matmul(
                        ps[32 * bc:32 * bc + MC, lo:hi],
                        w_sb[:, ci * MC:(ci + 1) * MC],
                        y_sb[:, ci * CW + bc * NFREE + lo:ci * CW + bc * NFREE + hi],
                        start=False,
                        stop=True,
                        tile_position=(0, 32 * bc),
                    )
                    mm.then_inc(pe_done, 1)

        @block.vector
        def _(vector):
            conv_ops(vector, "vector")
            vector.wait_ge(sem_s, 16)
            for half in range(2):
                lo, hi = half * NFREE // 2, (half + 1) * NFREE // 2
                vector.wait_ge(pe_done, NB * (half + 1))
                vector.tensor_mul(
                    out=u_sb[:, lo:hi], in0=ps[:, lo:hi], in1=s_sb[:, lo:hi]
                ).then_inc(ve_done, 1)

    _CACHE["nc"] = nc
    return nc


def _ensure_ntff_hook():
    """bass_utils hard-imports antenv.axon_hooks when BASS_TRACE is set;
    this container's trimmed antenv lacks it.  Register a working stub
    built from trn_agent_boot's ctypes NTFF driver (or a None hook,
    which bass_utils degrades gracefully on)."""
    import importlib.util
    import sys
    import types

    if "antenv.axon_hooks" in sys.modules:
        return
    try:
        if importlib.util.find_spec("antenv.axon_hooks") is not None:
            return
    except (ImportError, ValueError):
        pass
    try:
        from trn_agent_boot.trn_boot import _ntff_profile_via_ctypes

        hook = _ntff_profile_via_ctypes("/opt/axon/libaxon_pjrt.so")
    except Exception:
        hook = None
    mod = types.ModuleType("antenv.axon_hooks")
    mod.get_axon_ntff_profile_hook = lambda: hook
    sys.modules["antenv.axon_hooks"] = mod


def kernel(y_rev, M0, M_tilde, M_0l, M_big, sigma, lambda_e, phi, phi_tilde):
    _ensure_ntff_hook()
    from concourse.bass_utils import run_bass_kernel_spmd

    W = _build_w(M0, M_tilde, M_0l, M_big, sigma, lambda_e, phi, phi_tilde)
    # W_flat[k, c] with k = d*P + p, then swizzled so chunk ki sits at
    # columns [ki*MC, (ki+1)*MC) of a [128, NKC*MC] tile.
    Wf = W.transpose(0, 2, 1).reshape(K, MC)
    Wd = np.ascontiguousarray(
        Wf.reshape(NKC, 128, MC).transpose(1, 0, 2).reshape(128, NKC * MC)
    ).astype(np.float16)

    in_maps = []
    for sh in range(NCORES):
        blk = y_rev[sh * BS:(sh + 1) * BS, :KD, :].reshape(BS, K)  # [b, k]
        srow = (np.abs(blk).max(axis=1) / 127.0).astype(np.float32)  # [BS]
        np.maximum(srow, 1e-30, out=srow)
        q = np.rint(blk / srow[:, None])
        np.clip(q, -127, 127, out=q)
        q = q.astype(np.int8)
        # partition-major DRAM layout: y8[p, ki*CW + j] = q[j, ki*128 + p]
        ytp = np.ascontiguousarray(
            q.T.reshape(NKC, 128, CW).transpose(1, 0, 2).reshape(128, NKC * CW)
        )
        stile = np.empty((128, NFREE), dtype=np.float32)
        for bc in range(NB):
            stile[32 * bc:32 * (bc + 1), :] = srow[None, bc * NFREE:(bc + 1) * NFREE]
        in_maps.append({"y8": ytp, "w": Wd, "s": stile})

    res = run_bass_kernel_spmd(_get_nc(), in_maps, list(range(NCORES)))
    _CACHE["last_result"] = res

    out = np.empty((B, MC), dtype=np.float32)
    for sh in range(NCORES):
        # ut[32*bc + c, j] = u^T[c, bc*512 + j]
        stripes = res.results[sh]["ut"].reshape(NB, 32, NFREE)[:, :MC, :]
        out[sh * BS:(sh + 1) * BS, :] = (
            stripes.transpose(0, 2, 1).reshape(BS, MC).astype(np.float32)
        )
    return out


# revision 16
# speedup vs baseline: 1.7899x; 1.0090x over previous
"""Trainium2 Bass kernel for nn_DSC_86071144612259.

The reference network collapses to a single linear contraction

    u[b, c] = sum_{d<128} sum_{p} W[d, p, c] * y_rev[b, d, p]

where W [128, P, MC] is assembled exactly (float64, on host) from the
small parameter tensors.  The 270 MB y_rev stream is the real work and
is purely DMA bound, so the kernel moves y as *int8* (per-batch-row
scale, absmax/127) -- half the HBM traffic of the fp16 baseline.  The
PE only eats float dtypes (the BIR verifier rejects integer matmuls),
so int8 y is upconverted to fp16 on-chip: the sync HWDGE ring streams
int8 (measured ~375 GB/s with 2 MB descriptors, the per-core HBM
share), and the casts are split between DVE tensor_copy (~1.22
us/chunk) and ACT activation-Copy (~2.0 us/chunk), which together
(~1.32 chunks/us) hide under the stream (~0.7 us/chunk).

The tensor engine chases per chunk with fp16 matmuls accumulating in
fp32 PSUM (4 batch blocks concurrently in disjoint 32-column PE
groups); the per-row dequant scale is applied by the final PSUM->SBUF
tensor_mul, fused with the output copy.  The only numeric loss is the
int8 rounding of y (measured absmax-rel ~9.2e-3 < the 2e-2 gate).

Sharding: pure data parallel over the batch axis across 8 cores (2048
rows each); W and the scale tile are replicated per-core inputs.
"""

import numpy as np

B = 16384      # batch
L = 129        # history length of y_rev
P = 32         # observation dim
MC = 16        # control dim (output)
H = 24         # spectral dim
M = 64         # filter length
NCORES = 8
BS = B // NCORES           # 2048 batch rows per core
KD = 128                   # delays with nonzero weight
K = KD * P                 # 4096 contraction length
NKC = K // 128             # 32 k-chunks of 128 partitions
CW = BS                    # SBUF columns per chunk (2048)
NFREE = 512                # matmul moving free dim (one fp32 PSUM bank)
NB = BS // NFREE           # 4 batch chunks per core

# All 32 chunks arrive int8 on the sync HWDGE ring and are cast to
# fp16 by DVE/ACT.  Group sizes ramp 1->8->1: fine granularity at the
# head (casts start on chunk 0 immediately) and tail (short critical
# path after the last byte), 2 MB descriptors in the bulk (the ring
# only reaches ~375 GB/s with large transfers).
SYNC_GROUPS = [[0], [1, 2], [3, 4, 5, 6], [7, 8, 9, 10, 11, 12, 13, 14],
               [15, 16, 17, 18, 19, 20, 21, 22], [23, 24, 25, 26],
               [27, 28], [29, 30], [31]]

# Cast runs: per arrival group, one contiguous run per engine, one
# tensor_copy/activation op per run (batching amortizes the ~150-290 ns
# per-op overhead).  Measured rates: DVE ~1067 ns/chunk (2x mode), ACT
# ~1707 ns/chunk -> split runs ~5:3.  GpSimd casts are NOT used: ~8
# us/chunk AND they drag concurrent DVE casts down to the same pace
# (measured).  GpSimd *casting DMAs* are also out: run concurrently
# with the plain HWDGE stream, both queues collapse (113+95 GB/s vs
# 375/212 solo).  Chunk 31 goes to DVE (faster) to shorten the tail.
CONV_RUNS = [
    ("vector", [0], 0),
    ("vector", [1], 1), ("scalar", [2], 1),
    ("vector", [3, 4, 5], 2), ("scalar", [6], 2),
    ("vector", [7, 8, 9, 10, 11], 3), ("scalar", [12, 13, 14], 3),
    ("vector", [15, 16, 17, 18, 19], 4), ("scalar", [20, 21, 22], 4),
    ("vector", [23, 24], 5), ("scalar", [25, 26], 5),
    ("vector", [27], 6), ("scalar", [28], 6),
    ("vector", [29], 7), ("scalar", [30], 7),
    ("vector", [31], 8),
]
CONV_ENGINES = ("vector", "scalar")
_CACHE = {}


def _build_w(M0, M_tilde, M_0l, M_big, sigma, lambda_e, phi, phi_tilde):
    """Collapse the parameter tensors into W [KD, MC, P] (float64).

    Mirrors reference.py exactly:
      term1: delay 0,      M0
      term2: delays 1..64, sum_i lambda_i^0.25 phi_tilde[j-1,i] M_tilde[i]
      term3: delays 0..63, sum_l sigma_l^0.25  phi[k,l]         M_0l[l]
      term4: delays 1..127 via conv(phi_tilde[:,i], phi[:,l]) and M_big
    """
    f8 = np.float64
    M0 = M0.astype(f8)
    M_tilde = M_tilde.astype(f8)
    M_0l = M_0l.astype(f8)
    M_big = M_big.astype(f8)
    sigma = sigma.astype(f8)
    lambda_e = lambda_e.astype(f8)
    phi = phi.astype(f8)
    phi_tilde = phi_tilde.astype(f8)

    W = np.zeros((KD, MC, P), dtype=f8)
    W[0] += M0
    pt = phi_tilde * (lambda_e ** 0.25)[None, :]
    W[1:M + 1] += np.einsum("ji,icp->jcp", pt, M_tilde)
    ps = phi * (sigma ** 0.25)[None, :]
    W[0:M] += np.einsum("kl,lcp->kcp", ps, M_0l)
    W4 = np.empty((H, H, 2 * M - 1), dtype=f8)
    for i in range(H):
        for l in range(H):
            W4[i, l] = np.convolve(phi_tilde[:, i], phi[:, l])
    scale = (lambda_e[:, None] * sigma[None, :]) ** 0.25
    W[1:2 * M] += np.einsum("ild,ilcp->dcp", W4 * scale[:, :, None], M_big)
    return W


def _get_nc():
    """Build the per-core Bass program (cached)."""
    if "nc" in _CACHE:
        return _CACHE["nc"]
    import concourse.bass as bass
    import concourse.mybir as mybir

    # per-chunk: (engine, run-ordinal on that engine) for matmul waits
    chunk_wait = {}
    runs_of = {e: [] for e in CONV_ENGINES}
    for ename, chunks, gi in CONV_RUNS:
        runs_of[ename].append((chunks, gi))
        for ci in chunks:
            chunk_wait[ci] = (ename, len(runs_of[ename]))
    assert sorted(chunk_wait) == list(range(NKC))

    nc = bass.Bass("TRN2", target_bir_lowering=False, enable_partition_id=False)
    y8 = nc.dram_tensor("y8", [128, NKC * CW], mybir.dt.int8, kind="ExternalInput")
    w = nc.dram_tensor("w", [128, NKC * MC], mybir.dt.float16, kind="ExternalInput")
    s = nc.dram_tensor("s", [128, NFREE], mybir.dt.float32, kind="ExternalInput")
    ut = nc.dram_tensor("ut", [128, NFREE], mybir.dt.float16, kind="ExternalOutput")

    y8_sb = nc.alloc_sbuf_tensor("y8_sb", [128, NKC * CW], mybir.dt.int8)
    y_sb = nc.alloc_sbuf_tensor("y_sb", [128, NKC * CW], mybir.dt.float16)
    # W pre-swizzled on host: w_sb[p, ki*MC + c] = W_flat[ki*128 + p, c]
    w_sb = nc.alloc_sbuf_tensor("w_sb", [128, NKC * MC], mybir.dt.float16)
    # Dequant tile: s_sb[32*bc + c, j] = s_row[bc*512 + j]
    s_sb = nc.alloc_sbuf_tensor("s_sb", [128, NFREE], mybir.dt.float32)
    # Output striped across partitions: row 32*bc + c holds u^T[c, bc*512+j]
    u_sb = nc.alloc_sbuf_tensor("u_sb", [128, NFREE], mybir.dt.float16)
    ps = nc.alloc_psum_tensor("ps", [128, NFREE], mybir.dt.float32)

    sem_sg = [nc.alloc_semaphore(f"sem_sg{g}") for g in range(len(SYNC_GROUPS))]
    sem_w = nc.alloc_semaphore("sem_w")
    sem_s = nc.alloc_semaphore("sem_s")
    sem_cv = {e: nc.alloc_semaphore(f"sem_cv_{e}") for e in CONV_ENGINES}
    pe_done = nc.alloc_semaphore("pe_done")
    ve_done = nc.alloc_semaphore("ve_done")
    odma = nc.alloc_semaphore("odma")

    def conv_ops(eng, ename):
        lastg = None
        for chunks, gi in runs_of[ename]:
            if gi != lastg:
                eng.wait_ge(sem_sg[gi], 16)
                lastg = gi
            lo, hi = chunks[0] * CW, (chunks[-1] + 1) * CW
            if ename == "scalar":
                op = eng.copy(out=y_sb[:, lo:hi], in_=y8_sb[:, lo:hi])
            else:
                op = eng.tensor_copy(out=y_sb[:, lo:hi], in_=y8_sb[:, lo:hi])
            op.then_inc(sem_cv[ename], 1)

    with nc.Block() as block:

        @block.sync
        def _(sync):
            for g, chunks in enumerate(SYNC_GROUPS):
                lo, hi = chunks[0] * CW, (chunks[-1] + 1) * CW
                sync.dma_start(
                    out=y8_sb[:, lo:hi], in_=y8[:, lo:hi]
                ).then_inc(sem_sg[g], 16)
            sync.wait_ge(ve_done, 1)
            sync.dma_start(
                out=ut[:, :NFREE // 2], in_=u_sb[:, :NFREE // 2]
            ).then_inc(odma, 16)
            sync.wait_ge(odma, 32)

        @block.gpsimd
        def _(gpsimd):
            # the dequant tile is only needed by the final tensor_mul;
            # park its DMA on the otherwise idle gpsimd SWDGE queue
            gpsimd.dma_start(out=s_sb[:, :], in_=s[:, :]).then_inc(sem_s, 16)

        @block.scalar
        def _(scalar):
            # W first (tensor engine blocks on it), then casts.
            scalar.dma_start(out=w_sb[:, :], in_=w[:, :]).then_inc(sem_w, 16)
            conv_ops(scalar, "scalar")
            scalar.wait_ge(ve_done, 2)
            scalar.dma_start(
                out=ut[:, NFREE // 2:], in_=u_sb[:, NFREE // 2:]
            ).then_inc(odma, 16)
            scalar.wait_ge(odma, 32)

        @block.tensor
        def _(tensor):
            tensor.wait_ge(sem_w, 16)

            def wait_chunk(ci):
                e, n = chunk_wait[ci]
                tensor.wait_ge(sem_cv[e], n)

            for ci in range(NKC - 1):
                wait_chunk(ci)
                for bc in range(NB):
                    tensor.matmul(
                        ps[32 * bc:32 * bc + MC, :],
                        w_sb[:, ci * MC:(ci + 1) * MC],
                        y_sb[:, ci * CW + bc * NFREE:ci * CW + (bc + 1) * NFREE],
                        start=(ci == 0),
                        stop=False,
                        tile_position=(0, 32 * bc),
                    )
            # Last chunk in two N=256 halves so the dequant+store of the
            # first half overlaps the second half's matmuls.
            ci = NKC - 1
            wait_chunk(ci)
            for half in range(2):
                lo, hi = half * NFREE // 2, (half + 1) * NFREE // 2
                for bc in range(NB):
                    mm = tensor.matmul(
                        ps[32 * bc:32 * bc + MC, lo:hi],
                        w_sb[:, ci * MC:(ci + 1) * MC],
                        y_sb[:, ci * CW + bc * NFREE + lo:ci * CW + bc * NFREE + hi],
                        start=False,
                        stop=True,
                        tile_position=(0, 32 * bc),
                    )
                    mm.then_inc(pe_done, 1)

        @block.vector
        def _(vector):
            conv_ops(vector, "vector")
            vector.wait_ge(sem_s, 16)
            for half in range(2):
                lo, hi = half * NFREE // 2, (half + 1) * NFREE // 2
                vector.wait_ge(pe_done, NB * (half + 1))
                vector.tensor_mul(
                    out=u_sb[:, lo:hi], in0=ps[:, lo:hi], in1=s_sb[:, lo:hi]
                ).then_inc(ve_done, 1)

    _CACHE["nc"] = nc
    return nc


def _ensure_ntff_hook():
    """bass_utils hard-imports antenv.axon_hooks when BASS_TRACE is set;
    this container's trimmed antenv lacks it.  Register a working stub
    built from trn_agent_boot's ctypes NTFF driver (or a None hook,
    which bass_utils degrades gracefully on)."""
    import importlib.util
    import sys
    import types

    if "antenv.axon_hooks" in sys.modules:
        return
    try:
        if importlib.util.find_spec("antenv.axon_hooks") is not None:
            return
    except (ImportError, ValueError):
        pass
    try:
        from trn_agent_boot.trn_boot import _ntff_profile_via_ctypes

        hook = _ntff_profile_via_ctypes("/opt/axon/libaxon_pjrt.so")
    except Exception:
        hook = None
    mod = types.ModuleType("antenv.axon_hooks")
    mod.get_axon_ntff_profile_hook = lambda: hook
    sys.modules["antenv.axon_hooks"] = mod


def kernel(y_rev, M0, M_tilde, M_0l, M_big, sigma, lambda_e, phi, phi_tilde):
    _ensure_ntff_hook()
    from concourse.bass_utils import run_bass_kernel_spmd

    W = _build_w(M0, M_tilde, M_0l, M_big, sigma, lambda_e, phi, phi_tilde)
    # W_flat[k, c] with k = d*P + p, then swizzled so chunk ki sits at
    # columns [ki*MC, (ki+1)*MC) of a [128, NKC*MC] tile.
    Wf = W.transpose(0, 2, 1).reshape(K, MC)
    Wd = np.ascontiguousarray(
        Wf.reshape(NKC, 128, MC).transpose(1, 0, 2).reshape(128, NKC * MC)
    ).astype(np.float16)

    in_maps = []
    for sh in range(NCORES):
        blk = y_rev[sh * BS:(sh + 1) * BS, :KD, :].reshape(BS, K)  # [b, k]
        srow = (np.abs(blk).max(axis=1) / 127.0).astype(np.float32)  # [BS]
        np.maximum(srow, 1e-30, out=srow)
        q = np.rint(blk / srow[:, None])
        np.clip(q, -127, 127, out=q)
        q = q.astype(np.int8)
        # partition-major DRAM layout: y8[p, ki*CW + j] = q[j, ki*128 + p]
        ytp = np.ascontiguousarray(
            q.T.reshape(NKC, 128, CW).transpose(1, 0, 2).reshape(128, NKC * CW)
        )
        stile = np.empty((128, NFREE), dtype=np.float32)
        for bc in range(NB):
            stile[32 * bc:32 * (bc + 1), :] = srow[None, bc * NFREE:(bc + 1) * NFREE]
        in_maps.append({"y8": ytp, "w": Wd, "s": stile})

    res = run_bass_kernel_spmd(_get_nc(), in_maps, list(range(NCORES)))
    _CACHE["last_result"] = res

    out = np.empty((B, MC), dtype=np.float32)
    for sh in range(NCORES):
        # ut[32*bc + c, j] = u^T[c, bc*512 + j]
        stripes = res.results[sh]["ut"].reshape(NB, 32, NFREE)[:, :MC, :]
        out[sh * BS:(sh + 1) * BS, :] = (
            stripes.transpose(0, 2, 1).reshape(BS, MC).astype(np.float32)
        )
    return out
